# revision 27
# baseline (speedup 1.0000x reference)
"""Graph-ODE (GCN message passing) Trainium2 kernel.

Problem: h0 = x @ W_fc + b_fc; 4 Euler steps of
  h <- h + 0.25 * relu(gcn2(relu(gcn1(h)))),  gcn(h) = (adj @ h) @ W + b
with B=32, N=4096, IN_DIM=64, H=128.

Strategy (8 NeuronCores, data-parallel over batch):
 - Each core owns 4 batches; adj (pre-transposed + tiled on host) and
   weights are replicated. No collectives.
 - SWAPPED aggregation dataflow: stationary = state V node-major m-tiles
   [m,128h] per batch (fp8 DoubleRow pairs), moving = adjT column chunks
   [m, 512n]. PSUM accumulates over 32 m-tiles and lands aggT = (adj@V)^T
   in [h, n] orientation directly -- no PE transposes at all.
 - Projection consumes aggT n-tiles as stationary with W as the moving
   operand: out = aggT^T @ W = (adj@V) @ W arrives node-major [n, h'],
   exactly the layout the next aggregation needs. The projection for
   chunk/batch (c,b) is emitted one slot late (software pipeline) so the
   PE never waits on the PSUM->SBUF cast of its own aggregation.
 - Aggregation matmuls run in fp8-e4m3 with perf_mode=DoubleRow (256-K
   virtual rows, ~2x bf16 throughput). adj is scaled by 4096 on the host
   so its entries sit in e4m3 normal range; the scale is folded back via
   W/4096 in the projection. Projections stay bf16; Euler state h stays
   fp32 in SBUF.
 - Step-0 layer-1 aggregates x directly (adj@(x@Wfc) = (adj@x)@Wfc with
   W_fc@W1 folded on the host) in the same swapped dataflow: TWO batches'
   64-wide features pack the full 128 stationary columns, so its agg
   instructions are full-width; the projection peels each batch half with
   a K=64 stationary. Phase-0 units (h0 = x@W_fc, single bf16 term --
   plenty for the 2e-2 tolerance) interleave between its slots so the PE
   has work while input streams fill.
 - adjT chunks 0-1 are SBUF-resident (loaded once at start on the gpsimd
   queue); only chunks 2-7 re-stream per layer (12MB instead of 16MB).
 - Measured: ~965-975 us HW exec (PE ~95.5% busy; the 512-free fp8-DR agg
   matmuls run at 220ns vs the 213ns silicon ideal, i.e. the aggregation
   phase sits at ~98% of the 157 TF/s/core fp8 peak), rel err 2.39e-3 vs
   the fp32 reference (gate 2e-2). Session history: 1067 us (transpose
   dataflow, 3-term p0) -> 986 (swapped dataflow) -> 978 (b-pair x-layer)
   -> ~972 (resident chunks). Known residual slack: ~20 us x-window
   cold-start (input streams simply haven't arrived yet -- residency can't
   fix it), ~12 us of 64-K matmul overhead (p0/x-proj), ~13 us runtime
   DMA-queue warmup before the first instruction.
"""
import sys

sys.path.insert(0, "/opt/trn_rl_repo")

import numpy as np
import ml_dtypes

import concourse.bass as bass
import concourse.mybir as mybir
import concourse.tile as tile
from concourse.bass_utils import run_bass_kernel_spmd

BF16 = mybir.dt.bfloat16
FP8 = mybir.dt.float8e4
F32 = mybir.dt.float32
ADJ_SCALE = 4096.0

B, N, IN_DIM, H = 32, 4096, 64, 128
N_CORES = 8
BL = B // N_CORES          # 4 batches per core
NT = N // 128              # 32 node tiles
CH = 8                     # adjT column chunks (swapped moving operand)
CW = N // CH               # 512 columns per chunk
FREE = BL * H              # 512 moving free dim (legacy dataflow)
STEP = 0.25
N_STEPS = 4


def _split_multiwait(nc):
    """This walrus build accepts only ONE sync-wait command per engine
    instruction (incl. drains). Hoist extra waits onto preceding
    single-wait InstNoOps on the same engine."""
    import bass_rust
    for fn in nc.m.functions:
        for blk in fn.blocks:
            out = []
            for inst in blk.instructions:
                si = inst.sync_info
                if (si is not None and si.on_wait and len(si.on_wait) > 1
                        and type(inst).__name__ not in (
                            "InstTensorLoad", "InstTensorSave", "InstTrigger")):
                    waits = list(si.on_wait)
                    for w in waits[:-1]:
                        out.append(mybir.InstNoOp(
                            name=nc.get_next_instruction_name(),
                            engine=inst.engine, ins=[], outs=[],
                            sync_info=bass_rust.SyncInfo(
                                on_wait=[w], on_update=[]),
                        ))
                    inst.sync_info = bass_rust.SyncInfo(
                        on_wait=[waits[-1]], on_update=list(si.on_update))
                out.append(inst)
            blk.instructions = out


def _build(with_bias, fp8=True):
    nc = bass.Bass()

    adt = FP8 if fp8 else BF16
    new_path = fp8 and not with_bias
    if not new_path:
        adjt = nc.dram_tensor("adjt", [NT, 128, NT, 128], adt, kind="ExternalInput")
    if new_path:
        adjc = nc.dram_tensor("adjc", [CH, 128, NT, CW], adt, kind="ExternalInput")
        xn8 = nc.dram_tensor("xn8", [128, NT, BL // 2, 2 * IN_DIM], FP8, kind="ExternalInput")
        wfc1 = nc.dram_tensor("wfc1", [2 * IN_DIM, H], BF16, kind="ExternalInput")
    xt_hi = nc.dram_tensor("xt_hi", [BL, IN_DIM, N], BF16, kind="ExternalInput")
    if not new_path:
        xt_lo = nc.dram_tensor("xt_lo", [BL, IN_DIM, N], BF16, kind="ExternalInput")
    wpack = nc.dram_tensor("wpack", [128, 640], BF16, kind="ExternalInput")
    if with_bias:
        b_fc = nc.dram_tensor("b_fc", [1, H], BF16, kind="ExternalInput")
        b1 = nc.dram_tensor("b1", [1, H], BF16, kind="ExternalInput")
        b2 = nc.dram_tensor("b2", [1, H], BF16, kind="ExternalInput")
        ones = nc.dram_tensor("ones", [1, H], BF16, kind="ExternalInput")
    out = nc.dram_tensor("out", [BL, N, H], F32, kind="ExternalOutput")

    relu = mybir.ActivationFunctionType.Relu
    XC = 2048  # phase-0 x chunk (columns)

    with tile.TileContext(nc) as tc:
        with tc.tile_pool(name="res", bufs=1) as res, \
             tc.tile_pool(name="wgt", bufs=1) as wgt, \
             tc.tile_pool(name="xs", bufs=3) as xs, \
             tc.tile_pool(name="adjs", bufs=2 if fp8 and not with_bias else 3) as adjs, \
             tc.tile_pool(name="work", bufs=3) as work, \
             tc.tile_pool(name="ps", bufs=2, space="PSUM") as ps, \
             tc.tile_pool(name="psagg", bufs=3, space="PSUM") as psagg:

            # --- resident state: h (fp32) and fp8/bf16 activations, layout
            # [p, nt, b, h] (node-major interleaved)
            Hsb = res.tile([128, NT, BL, H], F32, tag="Hsb")
            Hbf = res.tile([128, NT, BL, H], adt, tag="Hbf")
            Tbf = res.tile([128, NT, BL, H], adt, tag="Tbf")

            # --- constants
            wpack_t = wgt.tile([128, 640], BF16, tag="wpack")
            nc.sync.dma_start(wpack_t[:], wpack[:])
            w1_t = wpack_t[:, 0:128]
            w2_t = wpack_t[:, 128:256]
            id_t = wpack_t[:, 256:384]
            wfc_hi_t = wpack_t[0:IN_DIM, 384:512]
            wfc_lo_t = wpack_t[0:IN_DIM, 512:640]
            if new_path:
                wfc1_t = wgt.tile([2 * IN_DIM, H], BF16, tag="wfc1")
                nc.sync.dma_start(wfc1_t[:], wfc1[:])
            if with_bias:
                bfc_t = wgt.tile([1, H], BF16, tag="bfc")
                b1_t = wgt.tile([1, H], BF16, tag="b1")
                b2_t = wgt.tile([1, H], BF16, tag="b2")
                ones_t = wgt.tile([1, H], BF16, tag="ones")
                nc.sync.dma_start(bfc_t[:], b_fc[:])
                nc.sync.dma_start(b1_t[:], b1[:])
                nc.sync.dma_start(b2_t[:], b2[:])
                nc.sync.dma_start(ones_t[:], ones[:])

            # --- phase 0 unit emitter: h0 = x @ W_fc + b_fc for one
            # (chunk, batch). Single bf16 term in the new path (2e-2
            # tolerance); 3-term hi/lo split in the legacy path.
            def emit_p0_unit(off, clen, b):
                xh = xs.tile([IN_DIM, XC], BF16, tag="xh")
                nc.sync.dma_start(xh[:, :clen], xt_hi[b, :, bass.ds(off, clen)])
                if not new_path:
                    xl = xs.tile([IN_DIM, XC], BF16, tag="xl")
                    nc.scalar.dma_start(xl[:, :clen], xt_lo[b, :, bass.ds(off, clen)])
                for j in range(clen // 128):
                    nt = (off // 128) + j
                    pz = ps.tile([128, H], F32, tag="pz0" if new_path else "pz")
                    xhs = xh[:, bass.ts(j, 128)]
                    if new_path:
                        nc.tensor.matmul(pz[:], xhs, wfc_hi_t,
                                         start=True, stop=True)
                    else:
                        xls = xl[:, bass.ts(j, 128)]
                        nc.tensor.matmul(pz[:], xhs, wfc_hi_t,
                                         start=True, stop=False)
                        nc.tensor.matmul(pz[:], xls, wfc_hi_t,
                                         start=False, stop=False)
                        last = not with_bias
                        nc.tensor.matmul(pz[:], xhs, wfc_lo_t,
                                         start=False, stop=last)
                        if with_bias:
                            nc.tensor.matmul(pz[:], ones_t[:], bfc_t[:],
                                             start=False, stop=True)
                    nc.vector.tensor_copy(Hsb[:, nt, b, :], pz[:])
                    if not new_path:
                        nc.scalar.activation(
                            Hbf[:, nt, b, :], pz[:],
                            mybir.ActivationFunctionType.Copy)

            chunks = [(0, 512), (512, 1536)] + [
                (o, XC) for o in range(2048, N, XC)]
            p0units = [(off, clen, b) for (off, clen) in chunks
                       for b in range(BL)]

            # --- step0/layer1 via x: adj@(x@Wfc) = (adj@x)@Wfc -> aggregate
            # x (64 feats, half cost) and project with host-folded Wfc@W1.
            # Phase-0 units are interleaved between aggregation chains so PE
            # has work from the first microsecond while streams prefetch.
            if new_path:
                # Swapped-dataflow x-layer: stationary packs TWO batches'
                # 64-wide x features into the full 128 PE columns
                # (out partitions 0:64 = batch 2bp, 64:128 = batch 2bp+1);
                # moving = adjT column chunks shared across batches. aggT_x
                # lands [2b x feat, n] with no transposes; the projection
                # peels each batch half with a K=64 stationary and emits
                # node-major relu((adj@x)@WfcW1) = Tbf directly.
                # adjT chunks 0-1 are SBUF-resident: loaded once at start on
                # the otherwise-idle vector DMA queue, reused by all layers.
                # This cuts per-layer stream traffic 16MB -> 12MB; the
                # x-layer (2 PE slots per chunk vs the main layers' 4) was
                # DMA-starved without it.
                AdjR = res.tile([128, 2, NT, CW], adt, tag="AdjR")
                for cc in range(2):
                    for q in range(4):
                        nc.gpsimd.dma_start(
                            AdjR[:, cc, bass.ts(q, 8), :],
                            adjc[cc, :, bass.ts(q, 8), :])

                def get_chunk(c):
                    """Returns rhs-slice fn for adjT chunk c (resident or
                    freshly streamed through the 2-deep ring)."""
                    if c < 2:
                        return lambda mt2: AdjR[:, c, bass.ts(mt2, 2), :]
                    adjb = adjs.tile([128, NT, CW], adt, tag="adjc")
                    for q in range(4):
                        nc.sync.dma_start(adjb[:, bass.ts(q, 8), :],
                                          adjc[c, :, bass.ts(q, 8), :])
                    return lambda mt2: adjb[:, bass.ts(mt2, 2), :]

                emit_p0_unit(*p0units[0])
                emit_p0_unit(*p0units[1])
                ui = 2
                # [p, nt, bp, 2b x 64f]: b-pair features pre-merged so the
                # stationary slice lowers to the same [128, 2, 128] AP shape
                # as the main loop's V slices (4-D slices defeat the AP
                # dim-merge and make LDWEIGHTS stall the PE).
                X8 = res.tile([128, NT, BL // 2, 2 * IN_DIM], FP8, tag="X8")
                for c8 in range(16):
                    nc.scalar.dma_start(X8[:, bass.ts(c8, 2), :, :],
                                        xn8[:, bass.ts(c8, 2), :, :])

                xprev = [None]

                def flush_xproj():
                    if xprev[0] is None:
                        return
                    paggTs, pc, pbp = xprev[0]
                    xprev[0] = None
                    for b2 in range(2):
                        pz = ps.tile([128, 4, H], F32, tag="pz")
                        for t in range(4):
                            nc.tensor.matmul(
                                pz[:, t, :],
                                paggTs[bass.ds(64 * b2, 64), bass.ts(t, 128)],
                                wfc1_t[bass.ds(64 * b2, 64), :],
                                start=True, stop=True)
                        nc.scalar.activation(
                            Tbf[:, bass.ds(4 * pc, 4), 2 * pbp + b2, :],
                            pz[:], relu)

                for c in range(CH):
                    chunk = get_chunk(c)
                    for bp in range(BL // 2):
                        pa = psagg.tile([128, CW], F32, tag="pagg")
                        for mt2 in range(NT // 2):
                            nc.tensor.matmul(
                                pa[:],
                                X8[:, bass.ts(mt2, 2), bp, :],
                                chunk(mt2),
                                start=(mt2 == 0), stop=(mt2 == NT // 2 - 1),
                                perf_mode=mybir.MatmulPerfMode.DoubleRow)
                        aggTs = work.tile([128, CW], BF16, tag="aggT")
                        nc.vector.tensor_copy(aggTs[:], pa[:])
                        flush_xproj()
                        xprev[0] = (aggTs, c, bp)
                        if ui < len(p0units):
                            emit_p0_unit(*p0units[ui])
                            ui += 1
                flush_xproj()
                while ui < len(p0units):
                    emit_p0_unit(*p0units[ui])
                    ui += 1
            else:
                for u in p0units:
                    emit_p0_unit(*u)

            if new_path:
                # --- 4 Euler steps x 2 GCN layers, swapped dataflow.
                # prev holds the deferred projection unit so the PE always
                # has the next aggregation queued before a projection that
                # depends on a fresh PSUM->SBUF cast.
                prev = [None]

                def flush_proj():
                    if prev[0] is None:
                        return
                    aggTs, step, layer, c, b = prev[0]
                    prev[0] = None
                    W = w1_t if layer == 0 else w2_t
                    pz = ps.tile([128, 4, H], F32, tag="pz")
                    for t in range(4):
                        nc.tensor.matmul(pz[:, t, :], aggTs[:, bass.ts(t, 128)],
                                         W, start=True, stop=True)
                    nt0 = 4 * c
                    sl = bass.ds(nt0, 4)
                    if layer == 0:
                        nc.scalar.activation(Tbf[:, sl, b, :], pz[:], relu)
                    else:
                        tmp = work.tile([128, 4, H], F32, tag="tmp")
                        nc.scalar.activation(tmp[:], pz[:], relu, scale=STEP)
                        nc.vector.tensor_add(Hsb[:, sl, b, :],
                                             Hsb[:, sl, b, :], tmp[:])
                        if step == N_STEPS - 1:
                            eng = nc.sync if c >= CH - 1 else nc.gpsimd
                            for t in range(4):
                                eng.dma_start(out[b, bass.ts(nt0 + t, 128), :],
                                              Hsb[:, nt0 + t, b, :])
                        else:
                            nc.vector.tensor_copy(Hbf[:, sl, b, :],
                                                  Hsb[:, sl, b, :])

                for step in range(N_STEPS):
                    for layer in range(2):
                        if step == 0 and layer == 0:
                            continue
                        V = Hbf if layer == 0 else Tbf
                        for c in range(CH):
                            chunk = get_chunk(c)
                            for b in range(BL):
                                pa = psagg.tile([128, CW], F32, tag="pagg")
                                for mt2 in range(NT // 2):
                                    nc.tensor.matmul(
                                        pa[:], V[:, bass.ts(mt2, 2), b, :],
                                        chunk(mt2),
                                        start=(mt2 == 0),
                                        stop=(mt2 == NT // 2 - 1),
                                        perf_mode=mybir.MatmulPerfMode.DoubleRow)
                                    if mt2 == 3:
                                        # nest the deferred projection inside
                                        # this agg chain: its 60ns multiplies
                                        # sit between 220ns agg multiplies so
                                        # every LDWEIGHTS stays hidden
                                        flush_proj()
                                aggTs = work.tile([128, CW], BF16, tag="aggT")
                                nc.vector.tensor_copy(aggTs[:], pa[:])
                                prev[0] = (aggTs, step, layer, c, b)
                flush_proj()
            else:
                # --- legacy dataflow: 4 Euler steps x 2 GCN layers
                for step in range(N_STEPS):
                    for layer in range(2):
                        V = Hbf if layer == 0 else Tbf
                        W = w1_t if layer == 0 else w2_t
                        bias = None if not with_bias else (b1_t if layer == 0 else b2_t)
                        for nt in range(NT):
                            blk = adjs.tile([128, NT, 128], adt, tag="adjblk")
                            nc.sync.dma_start(blk[:], adjt[nt])
                            pa = psagg.tile([128, BL, H], F32, tag="pagg")
                            if fp8:
                                for mt2 in range(NT // 2):
                                    nc.tensor.matmul(
                                        pa[:], blk[:, bass.ts(mt2, 2), :],
                                        V[:, bass.ts(mt2, 2), :, :],
                                        start=(mt2 == 0), stop=(mt2 == NT // 2 - 1),
                                        perf_mode=mybir.MatmulPerfMode.DoubleRow)
                            else:
                                for mt in range(NT):
                                    nc.tensor.matmul(pa[:], blk[:, mt, :], V[:, mt, :, :],
                                                     start=(mt == 0), stop=(mt == NT - 1))
                            agg = work.tile([128, BL, H], BF16, tag="agg")
                            nc.vector.tensor_copy(agg[:], pa[:])
                            ptr = ps.tile([128, BL, 128], BF16, tag="ptr")
                            for b in range(BL):
                                nc.tensor.transpose(ptr[:, b, :], agg[:, b, :], id_t)
                            aggT = work.tile([128, BL, 128], BF16, tag="aggT0")
                            nc.scalar.activation(aggT[:], ptr[:],
                                                 mybir.ActivationFunctionType.Copy)
                            pz = ps.tile([128, BL, H], F32, tag="pz")
                            for b in range(BL):
                                nc.tensor.matmul(pz[:, b, :], aggT[:, b, :], W,
                                                 start=True, stop=bias is None)
                                if bias is not None:
                                    nc.tensor.matmul(pz[:, b, :], ones_t[:], bias[:],
                                                     start=False, stop=True)
                            if layer == 0:
                                nc.scalar.activation(Tbf[:, nt, :, :], pz[:], relu)
                            else:
                                tmp = work.tile([128, BL, H], F32, tag="tmp")
                                nc.scalar.activation(tmp[:], pz[:], relu, scale=STEP)
                                nc.vector.tensor_add(Hsb[:, nt, :, :],
                                                     Hsb[:, nt, :, :], tmp[:])
                                if step == N_STEPS - 1:
                                    eng = nc.sync if nt >= NT - 4 else nc.gpsimd
                                    for b in range(BL):
                                        eng.dma_start(
                                            out[b, bass.ts(nt, 128), :],
                                            Hsb[:, nt, b, :])
                                else:
                                    nc.vector.tensor_copy(Hbf[:, nt, :, :],
                                                          Hsb[:, nt, :, :])

    _split_multiwait(nc)
    return nc


_NC_CACHE = {}


def _get_nc(with_bias, fp8=True):
    key = (with_bias, fp8)
    if key not in _NC_CACHE:
        _NC_CACHE[key] = _build(with_bias, fp8)
    return _NC_CACHE[key]


def _bf(a):
    return np.ascontiguousarray(a.astype(ml_dtypes.bfloat16))


def _prep_in_maps(x, adj, W_fc, b_fc, W1, b1, W2, b2, fp8=True):
    x = np.asarray(x, dtype=np.float32)
    adj = np.asarray(adj, dtype=np.float32)
    W_fc = np.asarray(W_fc, dtype=np.float32)
    b_fc = np.asarray(b_fc, dtype=np.float32)
    W1 = np.asarray(W1, dtype=np.float32)
    b1 = np.asarray(b1, dtype=np.float32)
    W2 = np.asarray(W2, dtype=np.float32)
    b2 = np.asarray(b2, dtype=np.float32)

    with_bias = bool(np.any(b_fc) or np.any(b1) or np.any(b2))
    new_path = fp8 and not with_bias

    # host layout prep (replicated operands)
    At = adj.T
    if fp8:
        w1h, w2h = _bf(W1 / ADJ_SCALE), _bf(W2 / ADJ_SCALE)
    else:
        w1h, w2h = _bf(W1), _bf(W2)
    wfc_hi = W_fc.astype(ml_dtypes.bfloat16).astype(np.float32)
    wfc_lo = W_fc - wfc_hi
    wpack = np.zeros((128, 640), dtype=np.float32)
    wpack[:, 0:128] = w1h.astype(np.float32)
    wpack[:, 128:256] = w2h.astype(np.float32)
    wpack[:, 256:384] = np.eye(128, dtype=np.float32)
    wpack[0:IN_DIM, 384:512] = wfc_hi
    wpack[0:IN_DIM, 512:640] = wfc_lo
    shared = {
        "wpack": _bf(wpack),
    }
    if new_path:
        shared["adjc"] = np.ascontiguousarray(
            (At.reshape(NT, 128, CH, CW).transpose(2, 1, 0, 3)
             * ADJ_SCALE).astype(ml_dtypes.float8_e4m3))  # [c, p, mt, j]
        wfc1h = (W_fc @ W1) / ADJ_SCALE
        shared["wfc1"] = _bf(np.vstack([wfc1h, wfc1h]))
    else:
        adjt = np.ascontiguousarray(
            At.reshape(NT, 128, NT, 128).transpose(2, 1, 0, 3))  # [nt, p, mt, j]
        if fp8:
            adjt = np.ascontiguousarray(
                (adjt * ADJ_SCALE).astype(ml_dtypes.float8_e4m3))
        else:
            adjt = _bf(adjt)
        shared["adjt"] = adjt
    if with_bias:
        shared.update({
            "b_fc": _bf(b_fc.reshape(1, H)),
            "b1": _bf(b1.reshape(1, H)),
            "b2": _bf(b2.reshape(1, H)),
            "ones": np.ones((1, H), dtype=ml_dtypes.bfloat16),
        })

    in_maps = []
    for c in range(N_CORES):
        xs = x[c * BL:(c + 1) * BL]                 # [BL, N, IN_DIM]
        xt = np.ascontiguousarray(xs.transpose(0, 2, 1))  # [BL, IN_DIM, N]
        xt_hi = xt.astype(ml_dtypes.bfloat16)
        m = {**shared,
             "xt_hi": np.ascontiguousarray(xt_hi)}
        if not new_path:
            m["xt_lo"] = _bf(xt - xt_hi.astype(np.float32))
        if new_path:
            xn8 = xs.reshape(BL, NT, 128, IN_DIM).transpose(2, 1, 0, 3)
            xn8 = xn8.reshape(128, NT, BL // 2, 2 * IN_DIM)
            m["xn8"] = np.ascontiguousarray(xn8.astype(ml_dtypes.float8_e4m3))
        in_maps.append(m)
    return in_maps, with_bias


FP8_DEFAULT = True


def kernel(**inputs):
    in_maps, with_bias = _prep_in_maps(**inputs, fp8=FP8_DEFAULT)
    nc = _get_nc(with_bias, FP8_DEFAULT)
    res = run_bass_kernel_spmd(nc, in_maps, core_ids=list(range(N_CORES)))
    return np.concatenate([res.results[c]["out"] for c in range(N_CORES)], axis=0)


def run_traced(**inputs):
    in_maps, with_bias = _prep_in_maps(**inputs, fp8=FP8_DEFAULT)
    nc = _get_nc(with_bias, FP8_DEFAULT)
    return run_bass_kernel_spmd(nc, in_maps, core_ids=list(range(N_CORES)),
                                trace=True)
